# revision 20
# baseline (speedup 1.0000x reference)
"""Channel-attention kernel for Trainium2, SPMD across 8 NeuronCores.

Problem: x:[4,512,64,64] f32; q = wq@x+bq, k = wk@x+bk (Cq=64), v = wv@x+bv;
scores = q^T k -> [B,4096,4096]; attn = softmax(scores, -1);
out = v @ attn^T; y = gamma*out + x.

Sharding: 8 shards = 4 batches x 2 query-halves. Each core gets its batch's
x pre-rotated along the pixel axis so its 2048 queries sit in columns 0:2048
(softmax/AV are permutation-invariant over keys, so rotating keys/values is
harmless). This keeps the SPMD program identical on every core.

Per-core pipeline (v4 -- column-slab streamed, ACT-saturating):
  P1: x streams in as eight 512-pixel column slabs; slab 0's four channel
      blocks go out on four different queues (sync/gpsimd/scalar/vector) so
      the first QK chunk lands ~4us after the preamble, later slabs ride
      sync (blocks 0,1) + gpsimd (blocks 2,3).  Per slab: fp8 cast on DVE,
      QK-projection in fp8 DoubleRow, bias-add on ACT (rides between exps),
      V-projection (lagging two slabs, per-half PSUM drains on DVE), and
      group-0 score pairs + exp + a j-by-j denominator chain right behind.
      Weight transposes run on the PE straight from f32 (no bf16 casts).
  P2: three group slots.  Slot g streams scores(g+1)+exp(g+1) finely
      interleaved with the four AV(g) ct-chains in 4-matmul chunks (so the
      in-order PE queue never parks on a not-yet-ready score PSUM buffer),
      the denominator chain for g+1 rides j-by-j two pairs behind the exp
      stream, and the (g, ct) epilogue (av*gamma/d on DVE, +gamma*bv +x on
      DVE from the f32 x still in SBUF) trails each ct-chain with y DMAs on
      the idle sync/gpsimd queues.
  Tail: AV(3) + epilogue only.

Residual precision: x is kept in f32 (no bf16 round-trip), so the visible
error of the gamma*attn + x path is tiny; the attention path runs in fp8
with a fixed exp bias of -4.
"""

import numpy as np

import concourse.bass as bass
import concourse.bacc as bacc
import concourse.mybir as mybir
import concourse.tile as tile
from concourse import bass_utils, masks

B, C, W, H = 4, 512, 64, 64
N = W * H          # 4096 pixels
CQ = 64            # query/key channels
NH = N // 2        # 2048 queries per core
NCORES = 8
F32 = mybir.dt.float32
BF16 = mybir.dt.bfloat16
FP8E4 = mybir.dt.float8e4
FP8E5 = mybir.dt.float8e5
DR = mybir.MatmulPerfMode.DoubleRow
VPAD = 528   # fp8 vT pair stride, %16 == 0
AF = mybir.ActivationFunctionType
MUL = mybir.AluOpType.mult
ADD = mybir.AluOpType.add

NJ = 16            # key-tile pairs
N_G = NH // 512    # 4 query groups per core
NS = 8             # x column slabs of 512 pixels


def _emit(tc, x, wq, wk, wv, bqk, bv, gamma, y):
    nc = tc.nc

    with (
        tc.tile_pool(name="const", bufs=1) as const,
        tc.tile_pool(name="data", bufs=1) as data,
        tc.tile_pool(name="wstg", bufs=1) as wstg,
    ):
        xf = [data.tile([128, N], F32, tag=f"xf{r}", name=f"xf{r}")
              for r in range(4)]

        # ---- x in [128,1024] chunks over three DMA queues ----------------
        # Per-queue DMA streams are latency-bound near ~120GB/s for 256KB
        # transfers but ~250GB/s at 512KB+, so x moves in 1024-px chunks:
        # sync carries block0 (+b2c0) and the k2lo/q2hi copies, gpsimd
        # carries blocks 1,3, scalar carries weights then block2's tail.
        def xchunk(eng, r, c):
            lo = c * 1024
            eng.dma_start(xf[r][:, lo:lo + 1024], x[r * 128:(r + 1) * 128,
                                                    lo:lo + 1024])
        xchunk(nc.sync, 0, 0)
        xchunk(nc.sync, 2, 0)
        for c in range(4):
            xchunk(nc.gpsimd, 1, c)
            xchunk(nc.gpsimd, 3, c)
        bqk_s = const.tile([128, 1], F32, tag="bqk")
        bv_s = const.tile([1, C], F32, tag="bvs")
        g_s = const.tile([1, 1], F32, tag="gs")
        wqk_f = wstg.tile([128, C], F32, tag="wqkf")
        nc.scalar.dma_start(wqk_f[0:CQ, :], wq)
        nc.scalar.dma_start(wqk_f[CQ:128, :], wk)
        nc.scalar.dma_start(bqk_s[:], bqk)
        nc.scalar.dma_start(bv_s[:], bv)
        nc.scalar.dma_start(g_s[:], gamma)
        wvf = []
        for r in range(4):
            wf = wstg.tile([128, C], F32, tag=f"wvf{r}", name=f"wf{r}")
            nc.scalar.dma_start(wf[:], wv[r * 128:(r + 1) * 128, :])
            wvf.append(wf)
        for c in range(1, 4):
            xchunk(nc.scalar, 2, c)

        # ---- constants (gpsimd memsets, before its x triggers) -----------
        id_bf = const.tile([128, 128], BF16, tag="idb")
        masks.make_identity(nc, id_bf[:])
        id_f32 = const.tile([128, 128], F32, tag="idf")
        masks.make_identity(nc, id_f32[:])
        ones_f32 = const.tile([1, 128], F32, tag="ones")
        nc.gpsimd.memset(ones_f32[:], 1.0)
        nbias = const.tile([128, 1], F32, tag="nbias")
        nc.gpsimd.memset(nbias[:], -4.0)
        onesP = const.tile([128, 32], FP8E4, tag="onesP")
        nc.gpsimd.memset(onesP[:], 1.0)


        # ---- persistent data ---------------------------------------------
        xp = [data.tile([128, 2 * N], FP8E4, tag=f"xp{pc}", name=f"xp{pc}")
              for pc in range(2)]
        qkb = data.tile([128, N], BF16, tag="qkb")
        k2lo = data.tile([64, N], BF16, tag="k2lo")
        q2hi = data.tile([128, NH], BF16, tag="q2hi")
        vP = [data.tile([128, 2 * VPAD], FP8E4, tag=f"vP{j}", name=f"vP{j}")
              for j in range(NJ)]
        wqkT8 = [data.tile([128, 256], FP8E4, tag=f"wqkT8{pc}",
                           name=f"wqkT8{pc}")
                 for pc in range(2)]
        wvTp = [data.tile([128, 1024], FP8E4, tag=f"wvTp{pc}",
                          name=f"wvTp{pc}")
                for pc in range(2)]
        gones = const.tile([1, 128], F32, tag="gones")
        gammab = const.tile([128, 1], F32, tag="gammab")
        gbv = const.tile([128, 4], F32, tag="gbv")

        def alloc_expP(g):
            return [data.tile([128, 1024], FP8E5, tag=f"expP{j}",
                              name=f"expP{j}_{g}", bufs=2)
                    for j in range(NJ)]

        with (
            tc.tile_pool(name="psSC", bufs=2, space="PSUM") as psSC,
            tc.tile_pool(name="psD", bufs=1, space="PSUM") as psD,
        ):
            ones_ap = onesP[:].rearrange("p (i n) -> p i n", i=2)[:, :, 0:1]

            def score_pair(expP_list, g, j):
                mA, mB = 2 * j, 2 * j + 1
                ps = psSC.tile([128, 1024], F32, tag="sc",
                               name=f"ps{g}_{j}")
                nc.tensor.matmul(
                    ps[:, 0:512], k2lo[:, mA * 128:(mA + 1) * 128],
                    qkb[0:CQ, g * 512:(g + 1) * 512],
                    start=True, stop=True,
                )
                nc.tensor.matmul(
                    ps[:, 512:1024],
                    qkb[CQ:128, mB * 128:(mB + 1) * 128],
                    q2hi[CQ:128, g * 512:(g + 1) * 512],
                    start=True, stop=True,
                )
                nc.scalar.activation(expP_list[j][:], ps[:], AF.Exp,
                                     bias=nbias[:])

            def dn_link(dt, expP_list, j):
                nc.tensor.matmul(
                    dt[0:1, :], ones_ap,
                    expP_list[j][:].rearrange("p (i n) -> p i n", i=2),
                    start=(j == 0), stop=(j == NJ - 1), perf_mode=DR,
                )

            # ================= P1: slab-streamed prologue =================
            with (
                tc.tile_pool(name="psQK", bufs=1, space="PSUM") as psQK,
                tc.tile_pool(name="psV", bufs=2, space="PSUM") as psV,
                tc.tile_pool(name="vstg", bufs=4) as vstg,
            ):
                expP = alloc_expP(0)

                def v_pair(j):
                    # two key tiles.  PSUM is drained by on-chip DMA (f32,
                    # rides the idle sync/gpsimd queues) and the fp8 cast
                    # runs SBUF->SBUF on DVE in its fast 2x mode.
                    for half in range(2):
                        mt = 2 * j + half
                        ps = psV.tile([128, 512], F32, tag="v",
                                      name=f"vps{j}_{half}")
                        for pc in range(2):
                            lhx = xp[pc][:].rearrange(
                                "p (i n) -> p i n", i=2)[
                                :, :, mt * 128:(mt + 1) * 128]
                            wvr = wvTp[pc][:].rearrange(
                                "p (i n) -> p i n", i=2)
                            nc.tensor.matmul(
                                ps[:], lhx, wvr,
                                start=(pc == 0), stop=(pc == 1),
                                perf_mode=DR,
                            )
                        nc.vector.tensor_copy(
                            vP[j][:, half * VPAD:half * VPAD + 512], ps[:])

                def slab_front(s):
                    """fp8 casts (DVE) + fp8 DR QK + bias on ACT + splits"""
                    lo = s * 512
                    if s in (0, 2, 4):
                        nlo = (s // 2 + 1) * 1024
                        nc.sync.dma_start(xf[0][:, nlo:nlo + 1024],
                                          x[0:128, nlo:nlo + 1024])
                    for r in range(4):
                        nc.vector.tensor_copy(
                            xp[r // 2][:, (r % 2) * N + lo:
                                       (r % 2) * N + lo + 512],
                            xf[r][:, lo:lo + 512])
                    qps = psQK.tile([128, 512], F32, tag="qk",
                                    name=f"qps{s}")
                    for pc in range(2):
                        mv = xp[pc][:].rearrange(
                            "p (i n) -> p i n", i=2)[:, :, lo:lo + 512]
                        st = wqkT8[pc][:].rearrange(
                            "p (i n) -> p i n", i=2)
                        nc.tensor.matmul(qps[:], st, mv,
                                         start=(pc == 0), stop=(pc == 1),
                                         perf_mode=DR)
                    # bias-add + bf16 cast on ACT (rides between exps)
                    nc.scalar.activation(qkb[:, lo:lo + 512], qps[:],
                                         AF.Identity, bias=bqk_s[:])
                    nc.sync.dma_start(
                        k2lo[:, lo:lo + 512], qkb[CQ:128, lo:lo + 512])
                    if s < 4:
                        nc.sync.dma_start(
                            q2hi[CQ:128, lo:lo + 512],
                            qkb[0:CQ, lo:lo + 512])

                # wq/wk transposed straight from f32; ptq shares the psQK
                # "qk" tag so it must be allocated before qps(0)
                ptq = [psQK.tile([128, 256], F32, tag="qk",
                                 name=f"ptq{i}") for i in range(2)]
                for cc in range(4):
                    nc.tensor.transpose(
                        ptq[cc // 2][:, (cc % 2) * 128:(cc % 2) * 128 + 128],
                        wqk_f[:, cc * 128:(cc + 1) * 128], id_f32[:])
                for pc in range(2):
                    nc.vector.tensor_copy(wqkT8[pc][:], ptq[pc][:])

                # -- slab 0 --
                slab_front(0)
                score_pair(expP, 0, 0)
                score_pair(expP, 0, 1)

                # -- slab 1 + wv prep (PE transposes from f32; fp8 copies
                #    on DVE) + epilogue constants --
                slab_front(1)
                for cc in range(4):
                    pt = psV.tile([128, C], F32, tag="v", name=f"ptv{cc}")
                    for r in range(4):
                        nc.tensor.transpose(
                            pt[:, r * 128:(r + 1) * 128],
                            wvf[r][:, cc * 128:(cc + 1) * 128],
                            id_f32[:],
                        )
                    nc.vector.tensor_copy(
                        wvTp[cc // 2][:, (cc % 2) * 512:(cc % 2) * 512 + 512],
                        pt[:])
                nc.vector.tensor_scalar_mul(gones[:], ones_f32[:], g_s[:])
                pg = psD.tile([128, 4], F32, tag="d", name="pg")
                nc.tensor.matmul(pg[:, 0:1], ones_f32[:], g_s[:],
                                 start=True, stop=True)
                nc.vector.tensor_copy(gammab[:], pg[:, 0:1])
                pbvT = psD.tile([128, 4], F32, tag="d", name="pbvT")
                for ct in range(4):
                    nc.tensor.matmul(
                        pbvT[:, ct:ct + 1],
                        bv_s[0:1, ct * 128:(ct + 1) * 128],
                        ones_f32[0:1, 0:1], start=True, stop=True)
                nc.vector.tensor_scalar_mul(gbv[:], pbvT[:], gammab[:])
                score_pair(expP, 0, 2)
                score_pair(expP, 0, 3)

                # -- slabs 2..7: steady state; v-pairs and the g0 denom
                #    chain lag two slabs/pairs behind --
                dt = psD.tile([128, 512], F32, tag="d", name="d0")
                for s in range(2, NS):
                    slab_front(s)
                    for j in (2 * s - 4, 2 * s - 3):
                        v_pair(j)
                    score_pair(expP, 0, 2 * s)
                    score_pair(expP, 0, 2 * s + 1)
                    dn_link(dt, expP, 2 * s - 4)
                    dn_link(dt, expP, 2 * s - 3)
                for j in (12, 13, 14, 15):
                    v_pair(j)
                    dn_link(dt, expP, j)

            # ============== P2: group slots + tail ========================
            with (
                tc.tile_pool(name="psAV", bufs=3, space="PSUM") as psAV,
                tc.tile_pool(name="small", bufs=2) as small,
                tc.tile_pool(name="yout", bufs=2) as yout,
            ):
                for g in range(N_G):
                    nxt = alloc_expP(g + 1) if g + 1 < N_G else None
                    dt_nxt = (psD.tile([128, 512], F32, tag="d",
                                       name=f"d{g + 1}")
                              if nxt is not None else None)
                    gcols = slice(g * 512, (g + 1) * 512)
                    dr = gdbs = av = None
                    for p in range(8):          # jj pairs
                        if nxt is not None:
                            score_pair(nxt, g + 1, 2 * p)
                            score_pair(nxt, g + 1, 2 * p + 1)
                            if p >= 1:
                                dn_link(dt_nxt, nxt, 2 * p - 2)
                                dn_link(dt_nxt, nxt, 2 * p - 1)
                        if p == 0:
                            # reciprocal runs on DVE hidden under the first
                            # AV half-chain; the gdb broadcast lands at p=1
                            dr = small.tile([1, 512], F32, tag="dr")
                            with nc.allow_low_precision(
                                    reason="approx 1/d; rescaled by gamma"):
                                nc.vector.reciprocal_approx_fast(
                                    dr[:], dt[0:1, :])
                        ct, half = p // 2, p % 2
                        if half == 0:
                            av = psAV.tile([128, 512], F32, tag="av",
                                           name=f"av{g}_{ct}")
                        for j in range(half * 8, half * 8 + 8):
                            vst = vP[j][:].rearrange(
                                "p (i n) -> p i n", i=2)[
                                :, :, ct * 128:(ct + 1) * 128]
                            nc.tensor.matmul(
                                av[:], vst,
                                expP[j][:].rearrange("p (i n) -> p i n",
                                                     i=2),
                                start=(j == 0), stop=(j == NJ - 1),
                                perf_mode=DR,
                            )
                        if p == 1:
                            gdb = psAV.tile([128, 512], F32, tag="av",
                                            name=f"gdb{g}")
                            nc.tensor.matmul(gdb[:], gones[:], dr[:],
                                             start=True, stop=True)
                            gdbs = small.tile([128, 512], F32, tag="gdbs",
                                              bufs=2)
                            nc.vector.tensor_copy(gdbs[:], gdb[:])
                        if half == 1:
                            tmp = yout.tile([128, 512], F32, tag="tmp")
                            nc.vector.tensor_tensor(tmp[:], av[:],
                                                    gdbs[:], MUL)
                            yo = yout.tile([128, 512], F32, tag="yo")
                            # yo = (tmp + gamma*bv) + x   (x f32 in SBUF)
                            nc.vector.scalar_tensor_tensor(
                                yo[:], tmp[:], gbv[:, ct:ct + 1],
                                xf[ct][:, gcols], ADD, ADD)
                            eng = nc.sync if ct % 2 == 0 else nc.gpsimd
                            eng.dma_start(
                                y[ct * 128:(ct + 1) * 128, gcols], yo[:])
                    if nxt is not None:
                        dn_link(dt_nxt, nxt, 14)
                        dn_link(dt_nxt, nxt, 15)
                    dt = dt_nxt
                    expP = nxt


def build_nc():
    nc = bacc.Bacc("TRN2", target_bir_lowering=False, debug=False,
                   num_devices=NCORES)
    x = nc.dram_tensor("x", [C, N], F32, kind="ExternalInput")
    wq = nc.dram_tensor("wq", [CQ, C], F32, kind="ExternalInput")
    wk = nc.dram_tensor("wk", [CQ, C], F32, kind="ExternalInput")
    wv = nc.dram_tensor("wv", [C, C], F32, kind="ExternalInput")
    bqk = nc.dram_tensor("bqk", [128, 1], F32, kind="ExternalInput")
    bv = nc.dram_tensor("bv", [1, C], F32, kind="ExternalInput")
    gamma = nc.dram_tensor("gamma", [1, 1], F32, kind="ExternalInput")
    y = nc.dram_tensor("y", [C, NH], F32, kind="ExternalOutput")
    with tile.TileContext(nc) as tc:
        _emit(tc, x.ap(), wq.ap(), wk.ap(), wv.ap(), bqk.ap(), bv.ap(),
              gamma.ap(), y.ap())
    nc.compile()
    return nc


def make_in_maps(inputs):
    xf = np.ascontiguousarray(
        np.asarray(inputs["x"], dtype=np.float32).reshape(B, C, N))
    wq = np.ascontiguousarray(np.asarray(inputs["wq"], dtype=np.float32))
    wk = np.ascontiguousarray(np.asarray(inputs["wk"], dtype=np.float32))
    wv = np.ascontiguousarray(np.asarray(inputs["wv"], dtype=np.float32))
    bqk = np.concatenate([
        np.asarray(inputs["bq"], dtype=np.float32),
        np.asarray(inputs["bk"], dtype=np.float32),
    ]).reshape(128, 1)
    bv = np.asarray(inputs["bv"], dtype=np.float32).reshape(1, C)
    gamma = np.asarray(inputs["gamma"], dtype=np.float32).reshape(1, 1)
    in_maps = []
    for i in range(NCORES):
        b, h = divmod(i, 2)
        xr = np.roll(xf[b], -h * NH, axis=1) if h else xf[b]
        in_maps.append({
            "x": np.ascontiguousarray(xr), "wq": wq, "wk": wk, "wv": wv,
            "bqk": bqk, "bv": bv, "gamma": gamma,
        })
    return in_maps


_NC = None


def _get_nc():
    global _NC
    if _NC is None:
        _NC = build_nc()
    return _NC


def kernel(**inputs):
    nc = _get_nc()
    in_maps = make_in_maps(inputs)
    res = bass_utils.run_bass_kernel_spmd(nc, in_maps, core_ids=list(range(NCORES)))
    yf = np.empty((B, C, N), dtype=np.float32)
    for i in range(NCORES):
        b, h = divmod(i, 2)
        yf[b][:, h * NH:(h + 1) * NH] = res.results[i]["y"]
    return yf.reshape(B, C, W, H)


# revision 21
# speedup vs baseline: 1.0624x; 1.0624x over previous
"""Channel-attention kernel for Trainium2, SPMD across 8 NeuronCores.

Problem: x:[4,512,64,64] f32; q = wq@x+bq, k = wk@x+bk (Cq=64), v = wv@x+bv;
scores = q^T k -> [B,4096,4096]; attn = softmax(scores, -1);
out = v @ attn^T; y = gamma*out + x.

Sharding: 8 shards = 4 batches x 2 query-halves. Each core gets its batch's
x pre-rotated along the pixel axis so its 2048 queries sit in columns 0:2048
(softmax/AV are permutation-invariant over keys, so rotating keys/values is
harmless). This keeps the SPMD program identical on every core.

Per-core pipeline (v4 -- column-slab streamed, ACT-saturating):
  P1: x streams in as eight 512-pixel column slabs; slab 0's four channel
      blocks go out on four different queues (sync/gpsimd/scalar/vector) so
      the first QK chunk lands ~4us after the preamble, later slabs ride
      sync (blocks 0,1) + gpsimd (blocks 2,3).  Per slab: fp8 cast on DVE,
      QK-projection in fp8 DoubleRow, bias-add on ACT (rides between exps),
      V-projection (lagging two slabs, per-half PSUM drains on DVE), and
      group-0 score pairs + exp + a j-by-j denominator chain right behind.
      Weight transposes run on the PE straight from f32 (no bf16 casts).
  P2: three group slots.  Slot g streams scores(g+1)+exp(g+1) finely
      interleaved with the four AV(g) ct-chains in 4-matmul chunks (so the
      in-order PE queue never parks on a not-yet-ready score PSUM buffer),
      the denominator chain for g+1 rides j-by-j two pairs behind the exp
      stream, and the (g, ct) epilogue (av*gamma/d on DVE, +gamma*bv +x on
      DVE from the f32 x still in SBUF) trails each ct-chain with y DMAs on
      the idle sync/gpsimd queues.
  Tail: AV(3) + epilogue only.

Residual precision: x is kept in f32 (no bf16 round-trip), so the visible
error of the gamma*attn + x path is tiny; the attention path runs in fp8
with a fixed exp bias of -4.
"""

import numpy as np

import concourse.bass as bass
import concourse.bacc as bacc
import concourse.mybir as mybir
import concourse.tile as tile
from concourse import bass_utils, masks

B, C, W, H = 4, 512, 64, 64
N = W * H          # 4096 pixels
CQ = 64            # query/key channels
NH = N // 2        # 2048 queries per core
NCORES = 8
F32 = mybir.dt.float32
BF16 = mybir.dt.bfloat16
FP8E4 = mybir.dt.float8e4
FP8E5 = mybir.dt.float8e5
DR = mybir.MatmulPerfMode.DoubleRow
VPAD = 528   # fp8 vT pair stride, %16 == 0
AF = mybir.ActivationFunctionType
MUL = mybir.AluOpType.mult
ADD = mybir.AluOpType.add

NJ = 16            # key-tile pairs
N_G = NH // 512    # 4 query groups per core
NS = 8             # x column slabs of 512 pixels


def _emit(tc, x, wq, wk, wv, bqk, bv, gamma, y):
    nc = tc.nc

    with (
        tc.tile_pool(name="const", bufs=1) as const,
        tc.tile_pool(name="data", bufs=1) as data,
        tc.tile_pool(name="wstg", bufs=1) as wstg,
    ):
        xf = [data.tile([128, N], F32, tag=f"xf{r}", name=f"xf{r}")
              for r in range(4)]

        # ---- constants first: gpsimd identities/memsets have no deps -----
        id_bf = const.tile([128, 128], BF16, tag="idb")
        masks.make_identity(nc, id_bf[:])
        id_f32 = const.tile([128, 128], F32, tag="idf")
        masks.make_identity(nc, id_f32[:])
        ones_f32 = const.tile([1, 128], F32, tag="ones")
        nc.gpsimd.memset(ones_f32[:], 1.0)
        nbias = const.tile([128, 1], F32, tag="nbias")
        nc.gpsimd.memset(nbias[:], -4.0)
        onesP = const.tile([128, 32], FP8E4, tag="onesP")
        nc.gpsimd.memset(onesP[:], 1.0)

        # ---- x: small slab-0 pieces for a fast start, then 1792-px
        # chunks (per-queue DMA streams are latency-bound for small
        # transfers, ~250GB/s for large ones) ------------------------------
        nc.sync.dma_start(xf[0][:, 0:512], x[0:128, 0:512])
        nc.gpsimd.dma_start(xf[1][:, 0:512], x[128:256, 0:512])
        nc.gpsimd.dma_start(xf[3][:, 0:512], x[384:512, 0:512])
        bqk_s = const.tile([128, 1], F32, tag="bqk")
        bv_s = const.tile([1, C], F32, tag="bvs")
        g_s = const.tile([1, 1], F32, tag="gs")
        wqk_f = wstg.tile([128, C], F32, tag="wqkf")
        nc.scalar.dma_start(wqk_f[0:CQ, :], wq)
        nc.scalar.dma_start(wqk_f[CQ:128, :], wk)
        nc.scalar.dma_start(xf[2][:, 0:512], x[256:384, 0:512])
        nc.scalar.dma_start(bqk_s[:], bqk)
        nc.scalar.dma_start(bv_s[:], bv)
        nc.scalar.dma_start(g_s[:], gamma)
        wvf = []
        for r in range(4):
            wf = wstg.tile([128, C], F32, tag=f"wvf{r}", name=f"wf{r}")
            nc.scalar.dma_start(wf[:], wv[r * 128:(r + 1) * 128, :])
            wvf.append(wf)

        def xtail(eng, r, c):
            lo = 512 + c * 1792
            eng.dma_start(xf[r][:, lo:lo + 1792],
                          x[r * 128:(r + 1) * 128, lo:lo + 1792])
        for c in range(2):
            xtail(nc.gpsimd, 1, c)
            xtail(nc.gpsimd, 3, c)
            xtail(nc.scalar, 2, c)


        # ---- persistent data ---------------------------------------------
        xp = [data.tile([128, 2 * N], FP8E4, tag=f"xp{pc}", name=f"xp{pc}")
              for pc in range(2)]
        qkb = data.tile([128, N], BF16, tag="qkb")
        k2lo = data.tile([64, N], BF16, tag="k2lo")
        q2hi = data.tile([128, NH], BF16, tag="q2hi")
        vP = [data.tile([128, 2 * VPAD], FP8E4, tag=f"vP{j}", name=f"vP{j}")
              for j in range(NJ)]
        wqkT8 = [data.tile([128, 256], FP8E4, tag=f"wqkT8{pc}",
                           name=f"wqkT8{pc}")
                 for pc in range(2)]
        wvTp = [data.tile([128, 1024], FP8E4, tag=f"wvTp{pc}",
                          name=f"wvTp{pc}")
                for pc in range(2)]
        gones = const.tile([1, 128], F32, tag="gones")
        gammab = const.tile([128, 1], F32, tag="gammab")
        gbv = const.tile([128, 4], F32, tag="gbv")

        def alloc_expP(g):
            return [data.tile([128, 1024], FP8E5, tag=f"expP{j}",
                              name=f"expP{j}_{g}", bufs=2)
                    for j in range(NJ)]

        with (
            tc.tile_pool(name="psSC", bufs=2, space="PSUM") as psSC,
            tc.tile_pool(name="psD", bufs=1, space="PSUM") as psD,
        ):
            ones_ap = onesP[:].rearrange("p (i n) -> p i n", i=2)[:, :, 0:1]

            def score_pair(expP_list, g, j):
                mA, mB = 2 * j, 2 * j + 1
                ps = psSC.tile([128, 1024], F32, tag="sc",
                               name=f"ps{g}_{j}")
                nc.tensor.matmul(
                    ps[:, 0:512], k2lo[:, mA * 128:(mA + 1) * 128],
                    qkb[0:CQ, g * 512:(g + 1) * 512],
                    start=True, stop=True,
                )
                nc.tensor.matmul(
                    ps[:, 512:1024],
                    qkb[CQ:128, mB * 128:(mB + 1) * 128],
                    q2hi[CQ:128, g * 512:(g + 1) * 512],
                    start=True, stop=True,
                )
                nc.scalar.activation(expP_list[j][:], ps[:], AF.Exp,
                                     bias=nbias[:])

            def dn_link(dt, expP_list, j):
                nc.tensor.matmul(
                    dt[0:1, :], ones_ap,
                    expP_list[j][:].rearrange("p (i n) -> p i n", i=2),
                    start=(j == 0), stop=(j == NJ - 1), perf_mode=DR,
                )

            # ================= P1: slab-streamed prologue =================
            with (
                tc.tile_pool(name="psQK", bufs=1, space="PSUM") as psQK,
                tc.tile_pool(name="psV", bufs=2, space="PSUM") as psV,
                tc.tile_pool(name="vstg", bufs=4) as vstg,
            ):
                expP = alloc_expP(0)

                def v_pair(j):
                    # two key tiles.  PSUM is drained by on-chip DMA (f32,
                    # rides the idle sync/gpsimd queues) and the fp8 cast
                    # runs SBUF->SBUF on DVE in its fast 2x mode.
                    for half in range(2):
                        mt = 2 * j + half
                        ps = psV.tile([128, 512], F32, tag="v",
                                      name=f"vps{j}_{half}")
                        for pc in range(2):
                            lhx = xp[pc][:].rearrange(
                                "p (i n) -> p i n", i=2)[
                                :, :, mt * 128:(mt + 1) * 128]
                            wvr = wvTp[pc][:].rearrange(
                                "p (i n) -> p i n", i=2)
                            nc.tensor.matmul(
                                ps[:], lhx, wvr,
                                start=(pc == 0), stop=(pc == 1),
                                perf_mode=DR,
                            )
                        nc.vector.tensor_copy(
                            vP[j][:, half * VPAD:half * VPAD + 512], ps[:])

                def slab_front(s):
                    """fp8 casts (DVE) + fp8 DR QK + bias on ACT + splits"""
                    lo = s * 512
                    if s in (0, 1):
                        nlo = 512 + s * 1792
                        nc.sync.dma_start(xf[0][:, nlo:nlo + 1792],
                                          x[0:128, nlo:nlo + 1792])
                    for r in range(4):
                        nc.vector.tensor_copy(
                            xp[r // 2][:, (r % 2) * N + lo:
                                       (r % 2) * N + lo + 512],
                            xf[r][:, lo:lo + 512])
                    qps = psQK.tile([128, 512], F32, tag="qk",
                                    name=f"qps{s}")
                    for pc in range(2):
                        mv = xp[pc][:].rearrange(
                            "p (i n) -> p i n", i=2)[:, :, lo:lo + 512]
                        st = wqkT8[pc][:].rearrange(
                            "p (i n) -> p i n", i=2)
                        nc.tensor.matmul(qps[:], st, mv,
                                         start=(pc == 0), stop=(pc == 1),
                                         perf_mode=DR)
                    # bias-add + bf16 cast on ACT (rides between exps)
                    nc.scalar.activation(qkb[:, lo:lo + 512], qps[:],
                                         AF.Identity, bias=bqk_s[:])
                    nc.sync.dma_start(
                        k2lo[:, lo:lo + 512], qkb[CQ:128, lo:lo + 512])
                    if s < 4:
                        nc.sync.dma_start(
                            q2hi[CQ:128, lo:lo + 512],
                            qkb[0:CQ, lo:lo + 512])

                # wq/wk transposed straight from f32; ptq shares the psQK
                # "qk" tag so it must be allocated before qps(0)
                ptq = [psQK.tile([128, 256], F32, tag="qk",
                                 name=f"ptq{i}") for i in range(2)]
                for cc in range(4):
                    nc.tensor.transpose(
                        ptq[cc // 2][:, (cc % 2) * 128:(cc % 2) * 128 + 128],
                        wqk_f[:, cc * 128:(cc + 1) * 128], id_f32[:])
                for pc in range(2):
                    nc.scalar.activation(wqkT8[pc][:], ptq[pc][:], AF.Copy)

                # -- slab 0 --
                slab_front(0)
                score_pair(expP, 0, 0)
                score_pair(expP, 0, 1)

                # -- slab 1 + wv prep (PE transposes from f32; fp8 copies
                #    on DVE) + epilogue constants --
                slab_front(1)
                for cc in range(4):
                    pt = psV.tile([128, C], F32, tag="v", name=f"ptv{cc}")
                    for r in range(4):
                        nc.tensor.transpose(
                            pt[:, r * 128:(r + 1) * 128],
                            wvf[r][:, cc * 128:(cc + 1) * 128],
                            id_f32[:],
                        )
                    nc.scalar.activation(
                        wvTp[cc // 2][:, (cc % 2) * 512:(cc % 2) * 512 + 512],
                        pt[:], AF.Copy)
                nc.vector.tensor_scalar_mul(gones[:], ones_f32[:], g_s[:])
                pg = psD.tile([128, 4], F32, tag="d", name="pg")
                nc.tensor.matmul(pg[:, 0:1], ones_f32[:], g_s[:],
                                 start=True, stop=True)
                nc.vector.tensor_copy(gammab[:], pg[:, 0:1])
                pbvT = psD.tile([128, 4], F32, tag="d", name="pbvT")
                for ct in range(4):
                    nc.tensor.matmul(
                        pbvT[:, ct:ct + 1],
                        bv_s[0:1, ct * 128:(ct + 1) * 128],
                        ones_f32[0:1, 0:1], start=True, stop=True)
                nc.vector.tensor_scalar_mul(gbv[:], pbvT[:], gammab[:])
                score_pair(expP, 0, 2)
                score_pair(expP, 0, 3)

                # -- slabs 2..7: steady state; v-pairs and the g0 denom
                #    chain lag two slabs/pairs behind --
                dt = psD.tile([128, 512], F32, tag="d", name="d0")
                for s in range(2, NS):
                    slab_front(s)
                    for j in (2 * s - 4, 2 * s - 3):
                        v_pair(j)
                    score_pair(expP, 0, 2 * s)
                    score_pair(expP, 0, 2 * s + 1)
                    dn_link(dt, expP, 2 * s - 4)
                    dn_link(dt, expP, 2 * s - 3)
                for j in (12, 13, 14, 15):
                    v_pair(j)
                    dn_link(dt, expP, j)

            # ============== P2: group slots + tail ========================
            with (
                tc.tile_pool(name="psAV", bufs=3, space="PSUM") as psAV,
                tc.tile_pool(name="small", bufs=2) as small,
                tc.tile_pool(name="yout", bufs=2) as yout,
            ):
                for g in range(N_G):
                    nxt = alloc_expP(g + 1) if g + 1 < N_G else None
                    dt_nxt = (psD.tile([128, 512], F32, tag="d",
                                       name=f"d{g + 1}")
                              if nxt is not None else None)
                    gcols = slice(g * 512, (g + 1) * 512)
                    dr = gdbs = av = None
                    for p in range(8):          # jj pairs
                        if nxt is not None:
                            score_pair(nxt, g + 1, 2 * p)
                            score_pair(nxt, g + 1, 2 * p + 1)
                            if p >= 1:
                                dn_link(dt_nxt, nxt, 2 * p - 2)
                                dn_link(dt_nxt, nxt, 2 * p - 1)
                        if p == 0:
                            # reciprocal runs on DVE hidden under the first
                            # AV half-chain; the gdb broadcast lands at p=1
                            dr = small.tile([1, 512], F32, tag="dr")
                            with nc.allow_low_precision(
                                    reason="approx 1/d; rescaled by gamma"):
                                nc.vector.reciprocal_approx_fast(
                                    dr[:], dt[0:1, :])
                        ct, half = p // 2, p % 2
                        if half == 0:
                            av = psAV.tile([128, 512], F32, tag="av",
                                           name=f"av{g}_{ct}")
                        for j in range(half * 8, half * 8 + 8):
                            vst = vP[j][:].rearrange(
                                "p (i n) -> p i n", i=2)[
                                :, :, ct * 128:(ct + 1) * 128]
                            nc.tensor.matmul(
                                av[:], vst,
                                expP[j][:].rearrange("p (i n) -> p i n",
                                                     i=2),
                                start=(j == 0), stop=(j == NJ - 1),
                                perf_mode=DR,
                            )
                        if p == 1:
                            gdb = psAV.tile([128, 512], F32, tag="av",
                                            name=f"gdb{g}")
                            nc.tensor.matmul(gdb[:], gones[:], dr[:],
                                             start=True, stop=True)
                            gdbs = small.tile([128, 512], F32, tag="gdbs",
                                              bufs=2)
                            nc.vector.tensor_copy(gdbs[:], gdb[:])
                        if half == 1:
                            tmp = yout.tile([128, 512], F32, tag="tmp")
                            nc.vector.tensor_tensor(tmp[:], av[:],
                                                    gdbs[:], MUL)
                            yo = yout.tile([128, 512], F32, tag="yo")
                            # yo = (tmp + gamma*bv) + x   (x f32 in SBUF)
                            nc.vector.scalar_tensor_tensor(
                                yo[:], tmp[:], gbv[:, ct:ct + 1],
                                xf[ct][:, gcols], ADD, ADD)
                            eng = nc.sync if ct % 2 == 0 else nc.gpsimd
                            eng.dma_start(
                                y[ct * 128:(ct + 1) * 128, gcols], yo[:])
                    if nxt is not None:
                        dn_link(dt_nxt, nxt, 14)
                        dn_link(dt_nxt, nxt, 15)
                    dt = dt_nxt
                    expP = nxt


def build_nc():
    nc = bacc.Bacc("TRN2", target_bir_lowering=False, debug=False,
                   num_devices=NCORES)
    x = nc.dram_tensor("x", [C, N], F32, kind="ExternalInput")
    wq = nc.dram_tensor("wq", [CQ, C], F32, kind="ExternalInput")
    wk = nc.dram_tensor("wk", [CQ, C], F32, kind="ExternalInput")
    wv = nc.dram_tensor("wv", [C, C], F32, kind="ExternalInput")
    bqk = nc.dram_tensor("bqk", [128, 1], F32, kind="ExternalInput")
    bv = nc.dram_tensor("bv", [1, C], F32, kind="ExternalInput")
    gamma = nc.dram_tensor("gamma", [1, 1], F32, kind="ExternalInput")
    y = nc.dram_tensor("y", [C, NH], F32, kind="ExternalOutput")
    with tile.TileContext(nc) as tc:
        _emit(tc, x.ap(), wq.ap(), wk.ap(), wv.ap(), bqk.ap(), bv.ap(),
              gamma.ap(), y.ap())
    nc.compile()
    return nc


def make_in_maps(inputs):
    xf = np.ascontiguousarray(
        np.asarray(inputs["x"], dtype=np.float32).reshape(B, C, N))
    wq = np.ascontiguousarray(np.asarray(inputs["wq"], dtype=np.float32))
    wk = np.ascontiguousarray(np.asarray(inputs["wk"], dtype=np.float32))
    wv = np.ascontiguousarray(np.asarray(inputs["wv"], dtype=np.float32))
    bqk = np.concatenate([
        np.asarray(inputs["bq"], dtype=np.float32),
        np.asarray(inputs["bk"], dtype=np.float32),
    ]).reshape(128, 1)
    bv = np.asarray(inputs["bv"], dtype=np.float32).reshape(1, C)
    gamma = np.asarray(inputs["gamma"], dtype=np.float32).reshape(1, 1)
    in_maps = []
    for i in range(NCORES):
        b, h = divmod(i, 2)
        xr = np.roll(xf[b], -h * NH, axis=1) if h else xf[b]
        in_maps.append({
            "x": np.ascontiguousarray(xr), "wq": wq, "wk": wk, "wv": wv,
            "bqk": bqk, "bv": bv, "gamma": gamma,
        })
    return in_maps


_NC = None


def _get_nc():
    global _NC
    if _NC is None:
        _NC = build_nc()
    return _NC


def kernel(**inputs):
    nc = _get_nc()
    in_maps = make_in_maps(inputs)
    res = bass_utils.run_bass_kernel_spmd(nc, in_maps, core_ids=list(range(NCORES)))
    yf = np.empty((B, C, N), dtype=np.float32)
    for i in range(NCORES):
        b, h = divmod(i, 2)
        yf[b][:, h * NH:(h + 1) * NH] = res.results[i]["y"]
    return yf.reshape(B, C, W, H)


# revision 23
# speedup vs baseline: 1.0710x; 1.0082x over previous
"""Channel-attention kernel for Trainium2, SPMD across 8 NeuronCores.

Problem: x:[4,512,64,64] f32; q = wq@x+bq, k = wk@x+bk (Cq=64), v = wv@x+bv;
scores = q^T k -> [B,4096,4096]; attn = softmax(scores, -1);
out = v @ attn^T; y = gamma*out + x.

Sharding: 8 shards = 4 batches x 2 query-halves. Each core gets its batch's
x pre-rotated along the pixel axis so its 2048 queries sit in columns 0:2048
(softmax/AV are permutation-invariant over keys, so rotating keys/values is
harmless). This keeps the SPMD program identical on every core.

Per-core pipeline (v4 -- column-slab streamed, ACT-saturating):
  P1: x streams in as eight 512-pixel column slabs; slab 0's four channel
      blocks go out on four different queues (sync/gpsimd/scalar/vector) so
      the first QK chunk lands ~4us after the preamble, later slabs ride
      sync (blocks 0,1) + gpsimd (blocks 2,3).  Per slab: fp8 cast on DVE,
      QK-projection in fp8 DoubleRow, bias-add on ACT (rides between exps),
      V-projection (lagging two slabs, per-half PSUM drains on DVE), and
      group-0 score pairs + exp + a j-by-j denominator chain right behind.
      Weight transposes run on the PE straight from f32 (no bf16 casts).
  P2: three group slots.  Slot g streams scores(g+1)+exp(g+1) finely
      interleaved with the four AV(g) ct-chains in 4-matmul chunks (so the
      in-order PE queue never parks on a not-yet-ready score PSUM buffer),
      the denominator chain for g+1 rides j-by-j two pairs behind the exp
      stream, and the (g, ct) epilogue (av*gamma/d on DVE, +gamma*bv +x on
      DVE from the f32 x still in SBUF) trails each ct-chain with y DMAs on
      the idle sync/gpsimd queues.
  Tail: AV(3) + epilogue only.

Residual precision: x is kept in f32 (no bf16 round-trip), so the visible
error of the gamma*attn + x path is tiny; the attention path runs in fp8
with a fixed exp bias of -4.
"""

import numpy as np

import concourse.bass as bass
import concourse.bacc as bacc
import concourse.mybir as mybir
import concourse.tile as tile
from concourse import bass_utils, masks

B, C, W, H = 4, 512, 64, 64
N = W * H          # 4096 pixels
CQ = 64            # query/key channels
NH = N // 2        # 2048 queries per core
NCORES = 8
F32 = mybir.dt.float32
BF16 = mybir.dt.bfloat16
FP8E4 = mybir.dt.float8e4
FP8E5 = mybir.dt.float8e5
DR = mybir.MatmulPerfMode.DoubleRow
VPAD = 528   # fp8 vT pair stride, %16 == 0
AF = mybir.ActivationFunctionType
MUL = mybir.AluOpType.mult
ADD = mybir.AluOpType.add

NJ = 16            # key-tile pairs
N_G = NH // 512    # 4 query groups per core
NS = 8             # x column slabs of 512 pixels


def _emit(tc, x, wqkT, wvT, bqk, bvT, gamma, y):
    nc = tc.nc

    with (
        tc.tile_pool(name="const", bufs=1) as const,
        tc.tile_pool(name="data", bufs=1) as data,
        tc.tile_pool(name="wstg", bufs=1) as wstg,
    ):
        xf = [data.tile([128, N], F32, tag=f"xf{r}", name=f"xf{r}")
              for r in range(4)]

        # ---- constants first: gpsimd memsets have no deps ----------------
        ones_f32 = const.tile([1, 128], F32, tag="ones")
        nc.gpsimd.memset(ones_f32[:], 1.0)
        nbias = const.tile([128, 1], F32, tag="nbias")
        nc.gpsimd.memset(nbias[:], -4.0)
        onesP = const.tile([128, 32], FP8E4, tag="onesP")
        nc.gpsimd.memset(onesP[:], 1.0)

        # ---- x: small slab-0 pieces for a fast start, then 1792-px
        # chunks (per-queue DMA streams are latency-bound for small
        # transfers, ~250GB/s for large ones) ------------------------------
        nc.sync.dma_start(xf[0][:, 0:512], x[0:128, 0:512])
        nc.gpsimd.dma_start(xf[1][:, 0:512], x[128:256, 0:512])
        nc.gpsimd.dma_start(xf[3][:, 0:512], x[384:512, 0:512])
        bqk_s = const.tile([128, 1], F32, tag="bqk")
        bvT_s = const.tile([128, 4], F32, tag="bvT")
        g_s = const.tile([1, 1], F32, tag="gs")
        wqkT_f = [wstg.tile([128, 256], F32, tag=f"wqkTf{pc}",
                            name=f"wqkTf{pc}") for pc in range(2)]
        for pc in range(2):
            for i in range(2):
                nc.scalar.dma_start(
                    wqkT_f[pc][:, i * 128:(i + 1) * 128],
                    wqkT[(2 * pc + i) * 128:(2 * pc + i + 1) * 128, :])
        nc.scalar.dma_start(xf[2][:, 0:512], x[256:384, 0:512])
        nc.scalar.dma_start(bqk_s[:], bqk)
        nc.scalar.dma_start(bvT_s[:], bvT)
        nc.scalar.dma_start(g_s[:], gamma)
        wvf = []
        for r in range(4):
            wf = wstg.tile([128, C], F32, tag=f"wvf{r}", name=f"wf{r}")
            nc.scalar.dma_start(wf[:], wvT[r * 128:(r + 1) * 128, :])
            wvf.append(wf)

        def xtail(eng, r, c):
            lo = 512 + c * 1792
            eng.dma_start(xf[r][:, lo:lo + 1792],
                          x[r * 128:(r + 1) * 128, lo:lo + 1792])
        for c in range(2):
            xtail(nc.gpsimd, 1, c)
            xtail(nc.gpsimd, 3, c)
            xtail(nc.scalar, 2, c)


        # ---- persistent data ---------------------------------------------
        xp = [data.tile([128, 2 * N], FP8E4, tag=f"xp{pc}", name=f"xp{pc}")
              for pc in range(2)]
        qkb = data.tile([128, N], BF16, tag="qkb")
        k2lo = data.tile([64, N], BF16, tag="k2lo")
        q2hi = data.tile([128, NH], BF16, tag="q2hi")
        vP = [data.tile([128, 2 * VPAD], FP8E4, tag=f"vP{j}", name=f"vP{j}")
              for j in range(NJ)]
        wqkT8 = [data.tile([128, 256], FP8E4, tag=f"wqkT8{pc}",
                           name=f"wqkT8{pc}")
                 for pc in range(2)]
        wvTp = [data.tile([128, 1024], FP8E4, tag=f"wvTp{pc}",
                          name=f"wvTp{pc}")
                for pc in range(2)]
        gones = const.tile([1, 128], F32, tag="gones")
        gammab = const.tile([128, 1], F32, tag="gammab")
        gbv = const.tile([128, 4], F32, tag="gbv")

        def alloc_expP(g):
            return [data.tile([128, 1024], FP8E5, tag=f"expP{j}",
                              name=f"expP{j}_{g}", bufs=2)
                    for j in range(NJ)]

        with (
            tc.tile_pool(name="psSC", bufs=2, space="PSUM") as psSC,
            tc.tile_pool(name="psD", bufs=1, space="PSUM") as psD,
        ):
            ones_ap = onesP[:].rearrange("p (i n) -> p i n", i=2)[:, :, 0:1]

            def score_pair(expP_list, g, j):
                mA, mB = 2 * j, 2 * j + 1
                ps = psSC.tile([128, 1024], F32, tag="sc",
                               name=f"ps{g}_{j}")
                nc.tensor.matmul(
                    ps[:, 0:512], k2lo[:, mA * 128:(mA + 1) * 128],
                    qkb[0:CQ, g * 512:(g + 1) * 512],
                    start=True, stop=True,
                )
                nc.tensor.matmul(
                    ps[:, 512:1024],
                    qkb[CQ:128, mB * 128:(mB + 1) * 128],
                    q2hi[CQ:128, g * 512:(g + 1) * 512],
                    start=True, stop=True,
                )
                nc.scalar.activation(expP_list[j][:], ps[:], AF.Exp,
                                     bias=nbias[:])

            def dn_link(dt, expP_list, j):
                nc.tensor.matmul(
                    dt[0:1, :], ones_ap,
                    expP_list[j][:].rearrange("p (i n) -> p i n", i=2),
                    start=(j == 0), stop=(j == NJ - 1), perf_mode=DR,
                )

            # ================= P1: slab-streamed prologue =================
            with (
                tc.tile_pool(name="psQK", bufs=1, space="PSUM") as psQK,
                tc.tile_pool(name="psV", bufs=2, space="PSUM") as psV,
                tc.tile_pool(name="vstg", bufs=4) as vstg,
            ):
                expP = alloc_expP(0)

                def v_pair(j):
                    # two key tiles.  PSUM is drained by on-chip DMA (f32,
                    # rides the idle sync/gpsimd queues) and the fp8 cast
                    # runs SBUF->SBUF on DVE in its fast 2x mode.
                    for half in range(2):
                        mt = 2 * j + half
                        ps = psV.tile([128, 512], F32, tag="v",
                                      name=f"vps{j}_{half}")
                        for pc in range(2):
                            lhx = xp[pc][:].rearrange(
                                "p (i n) -> p i n", i=2)[
                                :, :, mt * 128:(mt + 1) * 128]
                            wvr = wvTp[pc][:].rearrange(
                                "p (i n) -> p i n", i=2)
                            nc.tensor.matmul(
                                ps[:], lhx, wvr,
                                start=(pc == 0), stop=(pc == 1),
                                perf_mode=DR,
                            )
                        nc.vector.tensor_copy(
                            vP[j][:, half * VPAD:half * VPAD + 512], ps[:])

                def slab_front(s):
                    """fp8 casts (DVE) + fp8 DR QK + bias on ACT + splits"""
                    lo = s * 512
                    if s in (0, 1):
                        nlo = 512 + s * 1792
                        nc.sync.dma_start(xf[0][:, nlo:nlo + 1792],
                                          x[0:128, nlo:nlo + 1792])
                    for r in range(4):
                        nc.vector.tensor_copy(
                            xp[r // 2][:, (r % 2) * N + lo:
                                       (r % 2) * N + lo + 512],
                            xf[r][:, lo:lo + 512])
                    qps = psQK.tile([128, 512], F32, tag="qk",
                                    name=f"qps{s}")
                    for pc in range(2):
                        mv = xp[pc][:].rearrange(
                            "p (i n) -> p i n", i=2)[:, :, lo:lo + 512]
                        st = wqkT8[pc][:].rearrange(
                            "p (i n) -> p i n", i=2)
                        nc.tensor.matmul(qps[:], st, mv,
                                         start=(pc == 0), stop=(pc == 1),
                                         perf_mode=DR)
                    # bias-add + bf16 cast on ACT (rides between exps)
                    nc.scalar.activation(qkb[:, lo:lo + 512], qps[:],
                                         AF.Identity, bias=bqk_s[:])
                    nc.sync.dma_start(
                        k2lo[:, lo:lo + 512], qkb[CQ:128, lo:lo + 512])
                    if s < 4:
                        nc.sync.dma_start(
                            q2hi[CQ:128, lo:lo + 512],
                            qkb[0:CQ, lo:lo + 512])

                # weights arrive pre-transposed; just cast to fp8
                for pc in range(2):
                    nc.vector.tensor_copy(wqkT8[pc][:], wqkT_f[pc][:])
                for r in range(4):
                    nc.vector.tensor_copy(
                        wvTp[r // 2][:, (r % 2) * 512:(r % 2) * 512 + 512],
                        wvf[r][:])

                # -- slab 0 --
                slab_front(0)
                score_pair(expP, 0, 0)
                score_pair(expP, 0, 1)

                # -- slab 1 + epilogue constants --
                slab_front(1)
                nc.vector.tensor_scalar_mul(gones[:], ones_f32[:], g_s[:])
                pg = psD.tile([128, 4], F32, tag="d", name="pg")
                nc.tensor.matmul(pg[:, 0:1], ones_f32[:], g_s[:],
                                 start=True, stop=True)
                nc.vector.tensor_copy(gammab[:], pg[:, 0:1])
                nc.vector.tensor_scalar_mul(gbv[:], bvT_s[:], gammab[:])
                score_pair(expP, 0, 2)
                score_pair(expP, 0, 3)

                # -- slabs 2..7: steady state; v-pairs and the g0 denom
                #    chain lag two slabs/pairs behind --
                dt = psD.tile([128, 512], F32, tag="d", name="d0")
                for s in range(2, NS):
                    slab_front(s)
                    for j in (2 * s - 4, 2 * s - 3):
                        v_pair(j)
                    score_pair(expP, 0, 2 * s)
                    score_pair(expP, 0, 2 * s + 1)
                    dn_link(dt, expP, 2 * s - 4)
                    dn_link(dt, expP, 2 * s - 3)
                for j in (12, 13, 14, 15):
                    v_pair(j)
                    dn_link(dt, expP, j)

            # ============== P2: group slots + tail ========================
            with (
                tc.tile_pool(name="psAV", bufs=3, space="PSUM") as psAV,
                tc.tile_pool(name="small", bufs=2) as small,
                tc.tile_pool(name="yout", bufs=2) as yout,
            ):
                for g in range(N_G):
                    nxt = alloc_expP(g + 1) if g + 1 < N_G else None
                    dt_nxt = (psD.tile([128, 512], F32, tag="d",
                                       name=f"d{g + 1}")
                              if nxt is not None else None)
                    gcols = slice(g * 512, (g + 1) * 512)
                    dr = gdbs = av = None
                    for p in range(8):          # jj pairs
                        if nxt is not None:
                            score_pair(nxt, g + 1, 2 * p)
                            score_pair(nxt, g + 1, 2 * p + 1)
                            if p >= 1:
                                dn_link(dt_nxt, nxt, 2 * p - 2)
                                dn_link(dt_nxt, nxt, 2 * p - 1)
                        if p == 0:
                            # reciprocal runs on DVE hidden under the first
                            # AV half-chain; the gdb broadcast lands at p=1
                            dr = small.tile([1, 512], F32, tag="dr")
                            with nc.allow_low_precision(
                                    reason="approx 1/d; rescaled by gamma"):
                                nc.vector.reciprocal_approx_fast(
                                    dr[:], dt[0:1, :])
                        ct, half = p // 2, p % 2
                        if half == 0:
                            av = psAV.tile([128, 512], F32, tag="av",
                                           name=f"av{g}_{ct}")
                        for j in range(half * 8, half * 8 + 8):
                            vst = vP[j][:].rearrange(
                                "p (i n) -> p i n", i=2)[
                                :, :, ct * 128:(ct + 1) * 128]
                            nc.tensor.matmul(
                                av[:], vst,
                                expP[j][:].rearrange("p (i n) -> p i n",
                                                     i=2),
                                start=(j == 0), stop=(j == NJ - 1),
                                perf_mode=DR,
                            )
                        if p == 1:
                            gdb = psAV.tile([128, 512], F32, tag="av",
                                            name=f"gdb{g}")
                            nc.tensor.matmul(gdb[:], gones[:], dr[:],
                                             start=True, stop=True)
                            gdbs = small.tile([128, 512], F32, tag="gdbs",
                                              bufs=2)
                            nc.vector.tensor_copy(gdbs[:], gdb[:])
                        if half == 1:
                            tmp = yout.tile([128, 512], F32, tag="tmp")
                            nc.vector.tensor_tensor(tmp[:], av[:],
                                                    gdbs[:], MUL)
                            yo = yout.tile([128, 512], F32, tag="yo")
                            # yo = (tmp + gamma*bv) + x   (x f32 in SBUF)
                            nc.vector.scalar_tensor_tensor(
                                yo[:], tmp[:], gbv[:, ct:ct + 1],
                                xf[ct][:, gcols], ADD, ADD)
                            eng = nc.sync if ct % 2 == 0 else nc.gpsimd
                            eng.dma_start(
                                y[ct * 128:(ct + 1) * 128, gcols], yo[:])
                    if nxt is not None:
                        dn_link(dt_nxt, nxt, 14)
                        dn_link(dt_nxt, nxt, 15)
                    dt = dt_nxt
                    expP = nxt


def build_nc():
    nc = bacc.Bacc("TRN2", target_bir_lowering=False, debug=False,
                   num_devices=NCORES)
    x = nc.dram_tensor("x", [C, N], F32, kind="ExternalInput")
    wqkT = nc.dram_tensor("wqkT", [C, 128], F32, kind="ExternalInput")
    wvT = nc.dram_tensor("wvT", [C, C], F32, kind="ExternalInput")
    bqk = nc.dram_tensor("bqk", [128, 1], F32, kind="ExternalInput")
    bvT = nc.dram_tensor("bvT", [128, 4], F32, kind="ExternalInput")
    gamma = nc.dram_tensor("gamma", [1, 1], F32, kind="ExternalInput")
    y = nc.dram_tensor("y", [C, NH], F32, kind="ExternalOutput")
    with tile.TileContext(nc) as tc:
        _emit(tc, x.ap(), wqkT.ap(), wvT.ap(), bqk.ap(), bvT.ap(),
              gamma.ap(), y.ap())
    nc.compile()
    return nc


def make_in_maps(inputs):
    xf = np.ascontiguousarray(
        np.asarray(inputs["x"], dtype=np.float32).reshape(B, C, N))
    wq = np.asarray(inputs["wq"], dtype=np.float32)
    wk = np.asarray(inputs["wk"], dtype=np.float32)
    wqkT = np.ascontiguousarray(np.concatenate([wq, wk], axis=0).T)
    wvT = np.ascontiguousarray(
        np.asarray(inputs["wv"], dtype=np.float32).T)
    bqk = np.concatenate([
        np.asarray(inputs["bq"], dtype=np.float32),
        np.asarray(inputs["bk"], dtype=np.float32),
    ]).reshape(128, 1)
    bvT = np.ascontiguousarray(
        np.asarray(inputs["bv"], dtype=np.float32).reshape(4, 128).T)
    gamma = np.asarray(inputs["gamma"], dtype=np.float32).reshape(1, 1)
    in_maps = []
    for i in range(NCORES):
        b, h = divmod(i, 2)
        xr = np.roll(xf[b], -h * NH, axis=1) if h else xf[b]
        in_maps.append({
            "x": np.ascontiguousarray(xr), "wqkT": wqkT, "wvT": wvT,
            "bqk": bqk, "bvT": bvT, "gamma": gamma,
        })
    return in_maps


_NC = None


def _get_nc():
    global _NC
    if _NC is None:
        _NC = build_nc()
    return _NC


def kernel(**inputs):
    nc = _get_nc()
    in_maps = make_in_maps(inputs)
    res = bass_utils.run_bass_kernel_spmd(nc, in_maps, core_ids=list(range(NCORES)))
    yf = np.empty((B, C, N), dtype=np.float32)
    for i in range(NCORES):
        b, h = divmod(i, 2)
        yf[b][:, h * NH:(h + 1) * NH] = res.results[i]["y"]
    return yf.reshape(B, C, W, H)
